# revision 39
# baseline (speedup 1.0000x reference)
"""Sparse (segment + causal) GQA attention on 8 Trainium2 NeuronCores.

Problem: nn_AttentionOp_27719718928719
  query (2, 1024, 32, 128) f32, key/value (2, 1024, 8, 128) f32,
  decoder_segment_ids (2, 1024) i32 (sorted) -> out (2, 1024, 32, 128) f32

Sharding: core c owns kv-head c and its 4 GQA query heads, both batches.
Perfect shard (no replication): Q, K, V, and the output all split 8 ways, and
the compiled program is identical on every core (the block schedule depends
only on the segment ids, which all cores share).

Device algorithm, one unit per (batch b, 128-query block tj) with all 4 heads
fused along the free axis (512 wide everywhere):
  for each valid key block si (causal + segment overlap, host-computed):
    S^T[s, (h,t)] = K[si]^T Q      3 bf16 hi/lo matmuls -> fp32-grade logits
    S^T += bias(s) * 1(h,t)        rank-1 matmuls adding -60000 to keys
                                   outside a t-span's segment (masking on PE)
    P^T = exp(S^T)                 ACT, writes float32r directly to SBUF
    causal zero (diag blocks only) one Pool affine_select for all 4 heads
    outT[d, (h,t)] += V[si]' P^T   f32r matmul, PSUM-accumulated over si
    sums[1, (h,t)] += 1' P^T       f32r ones matmul
  stage PSUM -> SBUF (DVE), DMA out.
No softmax max-subtraction: logits are O(+-50) so exp stays in fp32 range and
exp(x)/sum(exp(x)) matches the reference's exp(x-max)/sum(exp(x-max)) exactly.
Host does the (cheap) normalization out/sums and all layout transposes.
"""

import numpy as np
import ml_dtypes

B, T, S, NQ, NKV, D = 2, 1024, 1024, 32, 8, 128
G = NQ // NKV
BLK = 128
NBLK = S // BLK  # 8
W = G * BLK  # 512: fused 4-head free width
N_CORES = 8
HLOC = NQ // N_CORES  # 4
MASK_BIAS = -60000.0

_compiled_cache = {}

# Test-only knobs (the grading path never sets these): when TRACE is true the
# SPMD run captures an NTFF profile into TRACE_DIR.
TRACE = False
TRACE_DIR = None

# QK product mode: "hilo" = 3 bf16 hi/lo matmuls (fp32-grade logits),
# "f32r" = single fp32r matmul (3x less PE time, ~30x more logit error).
QK_MODE = "f32r"


def _split_bf16(x):
    hi = x.astype(ml_dtypes.bfloat16)
    lo = (x - hi.astype(np.float32)).astype(ml_dtypes.bfloat16)
    return hi, lo


def _segment_structure(seg):
    """Block schedule for one batch's (sorted) segment ids.

    Returns (sched, bias_classes):
      sched[tj] = list of (si, bias_ops, diag) where bias_ops is a list of
        (a, b, cls) adding bias class `cls` to t-columns [a, b) of the block,
        and diag marks the causal in-block mask.
      bias_classes = list of np bool arrays [BLK]: True where the key row gets
        MASK_BIAS.
    """
    seg = np.asarray(seg)
    t_idx = np.arange(S)
    seg_start = np.zeros(S, np.int64)
    seg_end = np.zeros(S, np.int64)
    for v in np.unique(seg):
        m = seg == v
        lo, hi = np.argmax(m), S - np.argmax(m[::-1])
        seg_start[m], seg_end[m] = lo, hi
    valid_ts = (t_idx[None, :] <= t_idx[:, None]) & (seg[None, :] == seg[:, None])
    v4 = valid_ts.reshape(NBLK, BLK, NBLK, BLK)
    vblk = v4.any(axis=(1, 3))  # [tj, si]
    fblk = v4.all(axis=(1, 3))

    classes = []  # list of np.bool arrays
    cls_key = {}

    def class_id(mask_rows):
        key = mask_rows.tobytes()
        if key not in cls_key:
            cls_key[key] = len(classes)
            classes.append(mask_rows.copy())
        return cls_key[key]

    sched = []
    for tj in range(NBLK):
        entries = []
        sis = [si for si in range(NBLK) if vblk[tj, si]]
        assert sis == list(range(min(sis), max(sis) + 1))
        for si in sis:
            # valid t-span [ta, tb): t-columns with at least one valid key in
            # this block. Outside it every key row is masked, exp == 0.0
            # exactly, so the whole column is skipped in QK/exp/PV/sums.
            tvalid = v4[tj, :, si, :].any(axis=1)
            w = np.where(tvalid)[0]
            ta, tb = int(w[0]), int(w[-1]) + 1
            assert (np.diff(w) == 1).all(), "t-span not contiguous"
            bias_ops = []
            if not fblk[tj, si] and not (si == tj and _only_causal(v4, tj, si)):
                tcols = np.arange(tj * BLK, (tj + 1) * BLK)
                lo_rel = np.clip(seg_start[tcols] - si * BLK, 0, BLK)
                hi_rel = np.clip(seg_end[tcols] - si * BLK, 0, BLK)
                a = ta
                for i in range(ta + 1, tb + 1):
                    if i == tb or lo_rel[i] != lo_rel[a] or hi_rel[i] != hi_rel[a]:
                        lo, hi = int(lo_rel[a]), int(hi_rel[a])
                        rows = np.ones(BLK, dtype=bool)
                        rows[lo:hi] = False  # False -> keep
                        if rows.any():
                            bias_ops.append((a, i, class_id(rows)))
                        a = i
            entries.append((si, bias_ops, si == tj, ta, tb))
        # diag block first: it always spans the full t-range, so it carries
        # the start=True PSUM init for outp/sums; later (possibly t-sliced)
        # blocks accumulate into an already-initialized region.
        assert entries[-1][2], "diag must be the last ascending entry"
        entries = [entries[-1]] + entries[:-1]
        sched.append(entries)
    return sched, classes


def _only_causal(v4, tj, si):
    """True if block (tj, si)'s invalid entries are exactly the causal ones."""
    blk = v4[tj, :, si, :]  # [t, s]
    t = np.arange(BLK)[:, None] + tj * BLK
    s = np.arange(BLK)[None, :] + si * BLK
    return bool((blk == (s <= t)).all())


def _build_program(scheds, all_classes, qk_mode):
    """Build the SPMD Bass program. scheds/all_classes indexed by batch."""
    import concourse.bass as bass  # noqa: F401
    from concourse import bacc
    import concourse.mybir as mybir
    import concourse.tile as tile

    DT = mybir.dt
    F32R = DT.float32r
    QDT = DT.bfloat16 if qk_mode == "hilo" else F32R
    ncls = [len(c) for c in all_classes]
    nc = bacc.Bacc(None, target_bir_lowering=False, debug=False)

    qhi_d = nc.dram_tensor("qhi", [B, D, NBLK, HLOC, BLK], QDT, kind="ExternalInput").ap()
    khi_d = nc.dram_tensor("khi", [B, D, S], QDT, kind="ExternalInput").ap()
    if qk_mode == "hilo":
        qlo_d = nc.dram_tensor("qlo", [B, D, NBLK, HLOC, BLK], QDT, kind="ExternalInput").ap()
        klo_d = nc.dram_tensor("klo", [B, D, S], QDT, kind="ExternalInput").ap()
    v_d = nc.dram_tensor("v", [B, NBLK, BLK, D], DT.bfloat16, kind="ExternalInput").ap()
    NU = B * NBLK  # 16 (b, tj) units; each owns one partition row of sm_all
    ocols_d = nc.dram_tensor("ocols_in", [BLK, 2 * NU], DT.bfloat16, kind="ExternalInput").ap()
    nbias = max(1, sum(ncls))
    bias_d = nc.dram_tensor("bias_in", [1, nbias * BLK], DT.bfloat16, kind="ExternalInput").ap()
    outT_d = nc.dram_tensor(
        "outT", [B, NBLK, D, HLOC, BLK], DT.bfloat16, kind="ExternalOutput"
    ).ap()
    sums_d = nc.dram_tensor(
        "sums", [NU, HLOC, BLK], DT.float32, kind="ExternalOutput"
    ).ap()

    cls_base = [0, ncls[0]]  # class index offset per batch

    with tile.TileContext(nc) as tc:
        with (
            tc.tile_pool(name="const", bufs=1) as constp,
            tc.tile_pool(name="qkv", bufs=1) as qkv,
            tc.tile_pool(name="pt", bufs=4) as ptp,
            tc.tile_pool(name="stage", bufs=4) as stage,
            tc.tile_pool(name="ps_s", bufs=2, space="PSUM") as ps_s,
            tc.tile_pool(name="ps_o", bufs=3, space="PSUM") as ps_o,
            tc.tile_pool(name="ps_m", bufs=1, space="PSUM") as ps_m,
        ):
            # b=0 inputs first so compute can start while b=1 still loads
            k_hi = qkv.tile([D, B, S], QDT)
            v_t = qkv.tile([BLK, B, NBLK, D], DT.bfloat16)
            q_hi = qkv.tile([D, B, NBLK, HLOC, BLK], QDT)
            if qk_mode == "hilo":
                k_lo = qkv.tile([D, B, S], QDT)
                q_lo = qkv.tile([D, B, NBLK, HLOC, BLK], QDT)
            # ocols[:, 16-u:32-u] is an [128, 16] eye-column window (column u
            # all-ones, rest zero) -> the sums matmul for unit u lands its
            # column sums in partition row u of the shared sm_all tile.
            ocols = constp.tile([BLK, 2 * NU], DT.bfloat16)
            ones_bf = constp.tile([1, HLOC, BLK], DT.bfloat16)
            bias_t = constp.tile([1, nbias * BLK], DT.bfloat16)
            exp_bias = constp.tile([BLK, 1], mybir.dt.float32)
            # All 16 units' sums accumulate in one PSUM bank (one drain at
            # the very end); each unit contributes one 512-row matmul over
            # its DVE-accumulated ptacc instead of one matmul per block.
            sm_all = ps_m.tile([NU, HLOC, BLK], mybir.dt.float32)

            def load_q(b, lo_blk, hi_blk, eng=None):
                eng = eng or nc.sync
                sl = np.s_[lo_blk:hi_blk]
                if qk_mode == "hilo":
                    eng.dma_start(out=q_hi[:, b, sl], in_=qhi_d[b, :, sl])
                    eng.dma_start(out=q_lo[:, b, sl], in_=qlo_d[b, :, sl])
                else:
                    eng.dma_start(
                        out=q_hi[:, b, sl], in_=qhi_d[b, :, sl].bitcast(F32R)
                    )

            def load_k(b, lo_blk, hi_blk, eng):
                sl = np.s_[lo_blk * BLK:hi_blk * BLK]
                if qk_mode == "hilo":
                    eng.dma_start(out=k_hi[:, b, sl], in_=khi_d[b, :, sl])
                    eng.dma_start(out=k_lo[:, b, sl], in_=klo_d[b, :, sl])
                else:
                    eng.dma_start(
                        out=k_hi[:, b, sl], in_=khi_d[b, :, sl].bitcast(F32R)
                    )

            def load_v(b, lo_blk, hi_blk, eng):
                sl = np.s_[lo_blk:hi_blk]
                eng.dma_start(
                    out=v_t[:, b, sl],
                    in_=v_d[b, sl].rearrange("si p d -> p si d"),
                )

            # The first unit (b0, tj0) needs only k/q/v block 0; stage those
            # tiny gating chunks first, one per DMA-capable engine (sync,
            # scalar, gpsimd) so all three hit the DMA queues immediately.
            # Only b=0's working set is enqueued up front so it doesn't share
            # DMA bandwidth with b=1 data that isn't needed until much later;
            # b=1's loads are interleaved into b=0's compute stream below.
            load_q(0, 0, 1, nc.scalar)
            load_k(0, 0, 2, nc.sync)
            load_v(0, 0, 2, nc.gpsimd)
            load_k(0, 2, NBLK, nc.scalar)
            nc.sync.dma_start(out=bias_t, in_=bias_d)
            nc.sync.dma_start(out=ocols, in_=ocols_d)
            nc.vector.memset(ones_bf, 1.0)
            nc.vector.memset(exp_bias, -30.0)
            load_q(0, 1, 2)
            load_v(0, 2, NBLK, nc.gpsimd)
            load_q(0, 2, 4)
            load_q(0, 4, NBLK)
            # b=1 loads, issued on sync between b=0 unit outputs (see loop)
            deferred = {
                1: lambda: load_k(1, 0, NBLK, nc.sync),
                2: lambda: load_q(1, 0, 2),
                3: lambda: load_q(1, 2, 4),
                4: lambda: load_q(1, 4, 6),
                5: lambda: (load_q(1, 6, NBLK), load_v(1, 0, NBLK, nc.sync)),
            }

            i_unit = 0
            for b in range(B):
                # b=1 runs its lightest unit (tj0: diag only) last so the
                # final pipeline drain is as short as possible; b=0 keeps
                # ascending order to match the input DMA arrival order.
                tj_order = list(range(NBLK)) if b == 0 else list(range(1, NBLK)) + [0]
                for tj in tj_order:
                    if b == 0 and tj in deferred:
                        deferred[tj]()
                    entries = scheds[b][tj]
                    u = b * NBLK + tj
                    outp = ps_o.tile([D, HLOC, BLK], mybir.dt.float32)
                    n_e = len(entries)
                    ptacc = None

                    def qk_into(st_half, si, bias_ops, ta, tb):
                        kh = k_hi[:, b, si * BLK:(si + 1) * BLK]
                        qh = q_hi[:, b, tj]
                        # f32r matmuls below 256 moving rows run at 1/4
                        # rate; pad the QK t-span to >= 64 columns (4 heads
                        # x 64 = 256 rows). Padded columns compute garbage
                        # logits that exp/PV/sums never read.
                        qa, qb = ta, tb
                        if qk_mode != "hilo" and qb - qa < 64:
                            qa = max(0, min(ta, BLK - 64))
                            qb = qa + 64
                        qsl = np.s_[qa:qb]
                        last_qk = len(bias_ops) == 0
                        if qk_mode == "hilo":
                            kl = k_lo[:, b, si * BLK:(si + 1) * BLK]
                            ql = q_lo[:, b, tj]
                            nc.tensor.matmul(st_half[:, :, qsl], kh, qh[:, :, qsl],
                                             start=True, stop=False,
                                             skip_group_check=True)
                            nc.tensor.matmul(st_half[:, :, qsl], kh, ql[:, :, qsl],
                                             start=False, stop=False,
                                             skip_group_check=True)
                            nc.tensor.matmul(st_half[:, :, qsl], kl, qh[:, :, qsl],
                                             start=False, stop=last_qk,
                                             skip_group_check=True)
                        else:
                            nc.tensor.matmul(st_half[:, :, qsl], kh, qh[:, :, qsl],
                                             start=True, stop=last_qk,
                                             skip_group_check=True)
                        for bi, (a, e, cls) in enumerate(bias_ops):
                            cid = cls_base[b] + cls
                            nc.tensor.matmul(
                                st_half[:, :, a:e],
                                bias_t[:, cid * BLK:(cid + 1) * BLK],
                                ones_bf[:, :, :e - a],
                                start=False, stop=bi == len(bias_ops) - 1,
                                skip_group_check=True,
                            )

                    # two blocks share one 2-bank PSUM tile and ONE exp
                    # instruction: halves the scalar-engine instruction and
                    # semaphore count on the QK->exp->PV critical chain.
                    for pi in range(0, n_e, 2):
                        pair = entries[pi:pi + 2]
                        st2 = ps_s.tile([BLK, 2, HLOC, BLK], mybir.dt.float32)
                        pt2 = ptp.tile([BLK, 2, HLOC, BLK], DT.bfloat16)
                        for j, (si, bias_ops, diag, ta, tb) in enumerate(pair):
                            qk_into(st2[:, j], si, bias_ops, ta, tb)
                        ua = min(e[3] for e in pair)
                        ub = max(e[4] for e in pair)
                        # exp(x - 30): headroom against fp32 exp overflow
                        # for unlucky logit maxima; cancels in out/sums
                        # exactly. Over the pair's union t-span: columns
                        # belonging to only one half exp garbage in the
                        # other, which PV/sums never read.
                        nj = len(pair)
                        nc.scalar.activation(
                            out=pt2[:, :nj, :, ua:ub], in_=st2[:, :nj, :, ua:ub],
                            func=mybir.ActivationFunctionType.Exp,
                            bias=exp_bias,
                        )
                        for j, (si, bias_ops, diag, ta, tb) in enumerate(pair):
                            idx = pi + j
                            pt = pt2[:, j]
                            tsl = np.s_[ta:tb]
                            if diag:
                                # keep s <= t for every head: iota = -4x +
                                # h + 4y, >= 0 iff y >= x
                                nc.gpsimd.affine_select(
                                    out=pt, in_=pt,
                                    compare_op=mybir.AluOpType.is_ge,
                                    fill=0.0, base=0,
                                    pattern=[[1, HLOC], [HLOC, BLK]],
                                    channel_multiplier=-HLOC,
                                )
                            # diag runs first and spans the full t-range, so
                            # it initializes the PSUM region the t-sliced
                            # later blocks accumulate into.
                            nc.tensor.matmul(outp[:, :, tsl], v_t[:, b, si],
                                             pt[:, :, tsl],
                                             start=idx == 0, stop=idx == n_e - 1,
                                             skip_group_check=True)
                            # accumulate P on DVE so the PE pays one 512-row
                            # sums matmul per unit instead of one per block;
                            # the diag exp output doubles as the accumulator.
                            if idx == 0:
                                ptacc = pt
                            else:
                                nc.vector.tensor_add(
                                    ptacc[:, :, tsl], pt[:, :, tsl],
                                    ptacc[:, :, tsl]
                                )

                    nc.tensor.matmul(sm_all, ocols[:, NU - u:2 * NU - u], ptacc,
                                     start=i_unit == 0, stop=i_unit == NU - 1,
                                     skip_group_check=True)
                    i_unit += 1

                    o_sb = stage.tile([D, HLOC, BLK], DT.bfloat16)
                    nc.vector.tensor_copy(out=o_sb, in_=outp)
                    nc.sync.dma_start(out=outT_d[b, tj], in_=o_sb)
            s_sb = stage.tile([NU, HLOC, BLK], mybir.dt.float32)
            nc.vector.tensor_copy(out=s_sb, in_=sm_all)
            nc.sync.dma_start(out=sums_d, in_=s_sb)
    nc.compile()
    return nc


def kernel(query, key, value, decoder_segment_ids):
    from concourse.bass_utils import run_bass_kernel_spmd

    query = np.asarray(query, dtype=np.float32)
    key = np.asarray(key, dtype=np.float32)
    value = np.asarray(value, dtype=np.float32)
    seg = np.asarray(decoder_segment_ids, dtype=np.int32)

    structs = [_segment_structure(seg[b]) for b in range(B)]
    scheds = [s[0] for s in structs]
    all_classes = [s[1] for s in structs]
    sig = tuple(
        tuple(tuple((si, tuple(ops), diag, ta, tb)
                    for (si, ops, diag, ta, tb) in entries)
              for entries in sched)
        for sched in scheds
    ) + tuple(c.tobytes() for cl in all_classes for c in cl) + (QK_MODE,)
    nc = _compiled_cache.get(sig)
    if nc is None:
        nc = _build_program(scheds, all_classes, QK_MODE)
        _compiled_cache[sig] = nc

    NU = B * NBLK
    ocols_in = np.zeros((BLK, 2 * NU), dtype=ml_dtypes.bfloat16)
    ocols_in[:, NU] = 1.0
    nbias = max(1, sum(len(c) for c in all_classes))
    bias_in = np.zeros((1, nbias * BLK), dtype=ml_dtypes.bfloat16)
    i = 0
    for cl in all_classes:
        for rows in cl:
            bias_in[0, i * BLK:(i + 1) * BLK] = np.where(rows, MASK_BIAS, 0.0)
            i += 1

    in_maps = []
    for c in range(N_CORES):
        q_c = query[:, :, c * HLOC:(c + 1) * HLOC, :]  # (B, T, HLOC, D)
        # -> (B, D, NBLK, HLOC, BLK): element [b,d,tj,h,y] = q_c[b, tj*128+y, h, d]
        qT = np.ascontiguousarray(
            q_c.transpose(0, 3, 1, 2)  # (B, D, T, HLOC)
            .reshape(B, D, NBLK, BLK, HLOC)
            .transpose(0, 1, 2, 4, 3)
        )
        kT = np.ascontiguousarray(key[:, :, c, :].transpose(0, 2, 1))  # (B, D, S)
        v_c = np.ascontiguousarray(
            value[:, :, c, :].reshape(B, NBLK, BLK, D)
        ).astype(ml_dtypes.bfloat16)
        m = {"v": v_c, "ocols_in": ocols_in, "bias_in": bias_in}
        if QK_MODE == "hilo":
            m["qhi"], m["qlo"] = _split_bf16(qT)
            m["khi"], m["klo"] = _split_bf16(kT)
        else:
            m["qhi"], m["khi"] = qT, kT
        in_maps.append(m)

    kwargs = {}
    if TRACE:
        kwargs = dict(trace=True, tmpdir=TRACE_DIR)
    res = run_bass_kernel_spmd(nc, in_maps, core_ids=list(range(N_CORES)), **kwargs)
    kernel.last_results = res

    out = np.empty((B, T, NQ, D), dtype=np.float32)
    for c in range(N_CORES):
        outT = res.results[c]["outT"]  # (B, NBLK, D, HLOC, BLK) bf16
        sums = res.results[c]["sums"]  # (B, NBLK, HLOC, BLK) f32
        o = outT.astype(np.float32).reshape(B, NBLK, D, HLOC, BLK)
        s = sums.reshape(B, NBLK, HLOC, BLK)
        # out[b, tj*128+y, c*4+h, d] = o[b, tj, d, h, y] / s[b, tj, h, y]
        o = o.transpose(0, 1, 4, 3, 2).reshape(B, T, HLOC, D)
        s = s.transpose(0, 1, 3, 2).reshape(B, T, HLOC)
        out[:, :, c * HLOC:(c + 1) * HLOC, :] = o / s[:, :, :, None]
    return out



# revision 40
# speedup vs baseline: 1.0738x; 1.0738x over previous
"""Sparse (segment + causal) GQA attention on 8 Trainium2 NeuronCores.

Problem: nn_AttentionOp_27719718928719
  query (2, 1024, 32, 128) f32, key/value (2, 1024, 8, 128) f32,
  decoder_segment_ids (2, 1024) i32 (sorted) -> out (2, 1024, 32, 128) f32

Sharding: core c owns kv-head c and its 4 GQA query heads, both batches.
Perfect shard (no replication): Q, K, V, and the output all split 8 ways, and
the compiled program is identical on every core (the block schedule depends
only on the segment ids, which all cores share).

Device algorithm, one unit per (batch b, 128-query block tj) with all 4 heads
fused along the free axis (512 wide everywhere):
  for each valid key block si (causal + segment overlap, host-computed):
    S^T[s, (h,t)] = K[si]^T Q      3 bf16 hi/lo matmuls -> fp32-grade logits
    S^T += bias(s) * 1(h,t)        rank-1 matmuls adding -60000 to keys
                                   outside a t-span's segment (masking on PE)
    P^T = exp(S^T)                 ACT, writes float32r directly to SBUF
    causal zero (diag blocks only) one Pool affine_select for all 4 heads
    outT[d, (h,t)] += V[si]' P^T   f32r matmul, PSUM-accumulated over si
    sums[1, (h,t)] += 1' P^T       f32r ones matmul
  stage PSUM -> SBUF (DVE), DMA out.
No softmax max-subtraction: logits are O(+-50) so exp stays in fp32 range and
exp(x)/sum(exp(x)) matches the reference's exp(x-max)/sum(exp(x-max)) exactly.
Host does the (cheap) normalization out/sums and all layout transposes.
"""

import numpy as np
import ml_dtypes

B, T, S, NQ, NKV, D = 2, 1024, 1024, 32, 8, 128
G = NQ // NKV
BLK = 128
NBLK = S // BLK  # 8
W = G * BLK  # 512: fused 4-head free width
N_CORES = 8
HLOC = NQ // N_CORES  # 4
MASK_BIAS = -60000.0

_compiled_cache = {}

# Test-only knobs (the grading path never sets these): when TRACE is true the
# SPMD run captures an NTFF profile into TRACE_DIR.
TRACE = False
TRACE_DIR = None

# QK product mode: "hilo" = 3 bf16 hi/lo matmuls (fp32-grade logits),
# "f32r" = single fp32r matmul (3x less PE time, ~30x more logit error).
QK_MODE = "f32r"


def _split_bf16(x):
    hi = x.astype(ml_dtypes.bfloat16)
    lo = (x - hi.astype(np.float32)).astype(ml_dtypes.bfloat16)
    return hi, lo


def _segment_structure(seg):
    """Block schedule for one batch's (sorted) segment ids.

    Returns (sched, bias_classes):
      sched[tj] = list of (si, bias_ops, diag) where bias_ops is a list of
        (a, b, cls) adding bias class `cls` to t-columns [a, b) of the block,
        and diag marks the causal in-block mask.
      bias_classes = list of np bool arrays [BLK]: True where the key row gets
        MASK_BIAS.
    """
    seg = np.asarray(seg)
    t_idx = np.arange(S)
    seg_start = np.zeros(S, np.int64)
    seg_end = np.zeros(S, np.int64)
    for v in np.unique(seg):
        m = seg == v
        lo, hi = np.argmax(m), S - np.argmax(m[::-1])
        seg_start[m], seg_end[m] = lo, hi
    valid_ts = (t_idx[None, :] <= t_idx[:, None]) & (seg[None, :] == seg[:, None])
    v4 = valid_ts.reshape(NBLK, BLK, NBLK, BLK)
    vblk = v4.any(axis=(1, 3))  # [tj, si]
    fblk = v4.all(axis=(1, 3))

    classes = []  # list of np.bool arrays
    cls_key = {}

    def class_id(mask_rows):
        key = mask_rows.tobytes()
        if key not in cls_key:
            cls_key[key] = len(classes)
            classes.append(mask_rows.copy())
        return cls_key[key]

    sched = []
    for tj in range(NBLK):
        entries = []
        sis = [si for si in range(NBLK) if vblk[tj, si]]
        assert sis == list(range(min(sis), max(sis) + 1))
        for si in sis:
            # valid t-span [ta, tb): t-columns with at least one valid key in
            # this block. Outside it every key row is masked, exp == 0.0
            # exactly, so the whole column is skipped in QK/exp/PV/sums.
            tvalid = v4[tj, :, si, :].any(axis=1)
            w = np.where(tvalid)[0]
            ta, tb = int(w[0]), int(w[-1]) + 1
            assert (np.diff(w) == 1).all(), "t-span not contiguous"
            bias_ops = []
            if not fblk[tj, si] and not (si == tj and _only_causal(v4, tj, si)):
                tcols = np.arange(tj * BLK, (tj + 1) * BLK)
                lo_rel = np.clip(seg_start[tcols] - si * BLK, 0, BLK)
                hi_rel = np.clip(seg_end[tcols] - si * BLK, 0, BLK)
                a = ta
                for i in range(ta + 1, tb + 1):
                    if i == tb or lo_rel[i] != lo_rel[a] or hi_rel[i] != hi_rel[a]:
                        lo, hi = int(lo_rel[a]), int(hi_rel[a])
                        rows = np.ones(BLK, dtype=bool)
                        rows[lo:hi] = False  # False -> keep
                        if rows.any():
                            bias_ops.append((a, i, class_id(rows)))
                        a = i
            entries.append((si, bias_ops, si == tj, ta, tb))
        # diag block first: it always spans the full t-range, so it carries
        # the start=True PSUM init for outp/sums; later (possibly t-sliced)
        # blocks accumulate into an already-initialized region.
        assert entries[-1][2], "diag must be the last ascending entry"
        entries = [entries[-1]] + entries[:-1]
        sched.append(entries)
    return sched, classes


def _only_causal(v4, tj, si):
    """True if block (tj, si)'s invalid entries are exactly the causal ones."""
    blk = v4[tj, :, si, :]  # [t, s]
    t = np.arange(BLK)[:, None] + tj * BLK
    s = np.arange(BLK)[None, :] + si * BLK
    return bool((blk == (s <= t)).all())


def _build_program(scheds, all_classes, qk_mode):
    """Build the SPMD Bass program. scheds/all_classes indexed by batch."""
    import concourse.bass as bass  # noqa: F401
    from concourse import bacc
    import concourse.mybir as mybir
    import concourse.tile as tile

    DT = mybir.dt
    F32R = DT.float32r
    QDT = DT.bfloat16 if qk_mode == "hilo" else F32R
    ncls = [len(c) for c in all_classes]
    nc = bacc.Bacc(None, target_bir_lowering=False, debug=False)

    qhi_d = nc.dram_tensor("qhi", [B, D, NBLK, HLOC, BLK], QDT, kind="ExternalInput").ap()
    khi_d = nc.dram_tensor("khi", [B, D, S], QDT, kind="ExternalInput").ap()
    if qk_mode == "hilo":
        qlo_d = nc.dram_tensor("qlo", [B, D, NBLK, HLOC, BLK], QDT, kind="ExternalInput").ap()
        klo_d = nc.dram_tensor("klo", [B, D, S], QDT, kind="ExternalInput").ap()
    v_d = nc.dram_tensor("v", [B, NBLK, BLK, D], DT.bfloat16, kind="ExternalInput").ap()
    NU = B * NBLK  # 16 (b, tj) units; each owns one partition row of sm_all
    ocols_d = nc.dram_tensor("ocols_in", [BLK, 2 * NU], DT.bfloat16, kind="ExternalInput").ap()
    nbias = max(1, sum(ncls))
    bias_d = nc.dram_tensor("bias_in", [1, nbias * BLK], DT.bfloat16, kind="ExternalInput").ap()
    outT_d = nc.dram_tensor(
        "outT", [B, NBLK, D, HLOC, BLK], DT.bfloat16, kind="ExternalOutput"
    ).ap()
    sums_d = nc.dram_tensor(
        "sums", [NU, HLOC, BLK], DT.float32, kind="ExternalOutput"
    ).ap()

    cls_base = [0, ncls[0]]  # class index offset per batch

    with tile.TileContext(nc) as tc:
        with (
            tc.tile_pool(name="const", bufs=1) as constp,
            tc.tile_pool(name="qkv", bufs=1) as qkv,
            tc.tile_pool(name="pt", bufs=6) as ptp,
            tc.tile_pool(name="pacc", bufs=3) as paccp,
            tc.tile_pool(name="stage", bufs=4) as stage,
            tc.tile_pool(name="ps_s", bufs=4, space="PSUM") as ps_s,
            tc.tile_pool(name="ps_o", bufs=3, space="PSUM") as ps_o,
            tc.tile_pool(name="ps_m", bufs=1, space="PSUM") as ps_m,
        ):
            # b=0 inputs first so compute can start while b=1 still loads
            k_hi = qkv.tile([D, B, S], QDT)
            v_t = qkv.tile([BLK, B, NBLK, D], DT.bfloat16)
            q_hi = qkv.tile([D, B, NBLK, HLOC, BLK], QDT)
            if qk_mode == "hilo":
                k_lo = qkv.tile([D, B, S], QDT)
                q_lo = qkv.tile([D, B, NBLK, HLOC, BLK], QDT)
            # ocols[:, 16-u:32-u] is an [128, 16] eye-column window (column u
            # all-ones, rest zero) -> the sums matmul for unit u lands its
            # column sums in partition row u of the shared sm_all tile.
            ocols = constp.tile([BLK, 2 * NU], DT.bfloat16)
            ones_bf = constp.tile([1, HLOC, BLK], DT.bfloat16)
            bias_t = constp.tile([1, nbias * BLK], DT.bfloat16)
            exp_bias = constp.tile([BLK, 1], mybir.dt.float32)
            # All 16 units' sums accumulate in one PSUM bank (one drain at
            # the very end); each unit contributes one 512-row matmul over
            # its DVE-accumulated ptacc instead of one matmul per block.
            sm_all = ps_m.tile([NU, HLOC, BLK], mybir.dt.float32)

            def load_q(b, lo_blk, hi_blk, eng=None):
                eng = eng or nc.sync
                sl = np.s_[lo_blk:hi_blk]
                if qk_mode == "hilo":
                    eng.dma_start(out=q_hi[:, b, sl], in_=qhi_d[b, :, sl])
                    eng.dma_start(out=q_lo[:, b, sl], in_=qlo_d[b, :, sl])
                else:
                    eng.dma_start(
                        out=q_hi[:, b, sl], in_=qhi_d[b, :, sl].bitcast(F32R)
                    )

            def load_k(b, lo_blk, hi_blk, eng):
                sl = np.s_[lo_blk * BLK:hi_blk * BLK]
                if qk_mode == "hilo":
                    eng.dma_start(out=k_hi[:, b, sl], in_=khi_d[b, :, sl])
                    eng.dma_start(out=k_lo[:, b, sl], in_=klo_d[b, :, sl])
                else:
                    eng.dma_start(
                        out=k_hi[:, b, sl], in_=khi_d[b, :, sl].bitcast(F32R)
                    )

            def load_v(b, lo_blk, hi_blk, eng):
                sl = np.s_[lo_blk:hi_blk]
                eng.dma_start(
                    out=v_t[:, b, sl],
                    in_=v_d[b, sl].rearrange("si p d -> p si d"),
                )

            # The first unit (b0, tj0) needs only k/q/v block 0; stage those
            # tiny gating chunks first, one per DMA-capable engine (sync,
            # scalar, gpsimd) so all three hit the DMA queues immediately.
            # Only b=0's working set is enqueued up front so it doesn't share
            # DMA bandwidth with b=1 data that isn't needed until much later;
            # b=1's loads are interleaved into b=0's compute stream below.
            load_q(0, 0, 1, nc.scalar)
            load_k(0, 0, 2, nc.sync)
            load_v(0, 0, 2, nc.gpsimd)
            load_k(0, 2, NBLK, nc.scalar)
            nc.sync.dma_start(out=bias_t, in_=bias_d)
            nc.sync.dma_start(out=ocols, in_=ocols_d)
            nc.vector.memset(ones_bf, 1.0)
            nc.vector.memset(exp_bias, -30.0)
            load_q(0, 1, 2)
            load_v(0, 2, NBLK, nc.gpsimd)
            load_q(0, 2, 4)
            load_q(0, 4, NBLK)
            # b=1 loads, issued on sync between b=0 unit outputs (see loop)
            deferred = {
                1: lambda: load_k(1, 0, NBLK, nc.sync),
                2: lambda: load_q(1, 0, 2),
                3: lambda: load_q(1, 2, 4),
                4: lambda: load_q(1, 4, 6),
                5: lambda: (load_q(1, 6, NBLK), load_v(1, 0, NBLK, nc.sync)),
            }

            i_unit = 0
            for b in range(B):
                # b=1 runs its lightest unit (tj0: diag only) last so the
                # final pipeline drain is as short as possible; b=0 keeps
                # ascending order to match the input DMA arrival order.
                tj_order = list(range(NBLK)) if b == 0 else list(range(1, NBLK)) + [0]
                for tj in tj_order:
                    if b == 0 and tj in deferred:
                        deferred[tj]()
                    entries = scheds[b][tj]
                    u = b * NBLK + tj
                    outp = ps_o.tile([D, HLOC, BLK], mybir.dt.float32)
                    ptacc = paccp.tile([BLK, HLOC, BLK], DT.bfloat16)
                    n_e = len(entries)
                    for idx, (si, bias_ops, diag, ta, tb) in enumerate(entries):
                        st = ps_s.tile([BLK, HLOC, BLK], mybir.dt.float32)
                        kh = k_hi[:, b, si * BLK:(si + 1) * BLK]
                        qh = q_hi[:, b, tj]
                        tsl = np.s_[ta:tb]
                        # f32r matmuls below 256 moving rows run at 1/4 rate;
                        # pad the QK t-span to >= 64 columns (4 heads x 64 =
                        # 256 rows). Padded columns compute garbage logits
                        # that exp/PV/sums never read.
                        qa, qb = ta, tb
                        if qk_mode != "hilo" and qb - qa < 64:
                            qa = max(0, min(ta, BLK - 64))
                            qb = qa + 64
                        qsl = np.s_[qa:qb]
                        last_qk = len(bias_ops) == 0
                        if qk_mode == "hilo":
                            kl = k_lo[:, b, si * BLK:(si + 1) * BLK]
                            ql = q_lo[:, b, tj]
                            nc.tensor.matmul(st[:, :, tsl], kh, qh[:, :, tsl],
                                             start=True, stop=False,
                                             skip_group_check=True)
                            nc.tensor.matmul(st[:, :, tsl], kh, ql[:, :, tsl],
                                             start=False, stop=False,
                                             skip_group_check=True)
                            nc.tensor.matmul(st[:, :, tsl], kl, qh[:, :, tsl],
                                             start=False, stop=last_qk,
                                             skip_group_check=True)
                        else:
                            nc.tensor.matmul(st[:, :, qsl], kh, qh[:, :, qsl],
                                             start=True, stop=last_qk,
                                             skip_group_check=True)
                        for bi, (a, e, cls) in enumerate(bias_ops):
                            cid = cls_base[b] + cls
                            nc.tensor.matmul(
                                st[:, :, a:e],
                                bias_t[:, cid * BLK:(cid + 1) * BLK],
                                ones_bf[:, :, :e - a],
                                start=False, stop=bi == len(bias_ops) - 1,
                                skip_group_check=True,
                            )

                        # exp(x - 30): headroom against fp32 exp overflow for
                        # unlucky logit maxima; cancels in out/sums exactly.
                        # The diag block (always first, always full-width)
                        # writes straight into ptacc, which doubles as its
                        # pt: no init copy needed.
                        if idx == 0:
                            pt = ptacc
                        else:
                            pt = ptp.tile([BLK, HLOC, BLK], DT.bfloat16)
                        nc.scalar.activation(
                            out=pt[:, :, tsl], in_=st[:, :, tsl],
                            func=mybir.ActivationFunctionType.Exp,
                            bias=exp_bias,
                        )
                        if diag:
                            # keep s <= t for every head: iota = -4x + h + 4y,
                            # >= 0 iff y >= x (h in 0..3 can't flip it)
                            nc.gpsimd.affine_select(
                                out=pt, in_=pt, compare_op=mybir.AluOpType.is_ge,
                                fill=0.0, base=0,
                                pattern=[[1, HLOC], [HLOC, BLK]],
                                channel_multiplier=-HLOC,
                            )

                        # diag runs first and spans the full t-range, so it
                        # initializes the PSUM region the t-sliced later
                        # blocks accumulate into.
                        nc.tensor.matmul(outp[:, :, tsl], v_t[:, b, si],
                                         pt[:, :, tsl],
                                         start=idx == 0, stop=idx == n_e - 1,
                                         skip_group_check=True)
                        # accumulate P on DVE (bf16 2x mode) so the PE pays
                        # one 512-row sums matmul per unit instead of one
                        # per block; the diag exp already initialized ptacc.
                        if idx > 0:
                            nc.vector.tensor_add(
                                ptacc[:, :, tsl], pt[:, :, tsl], ptacc[:, :, tsl]
                            )

                    nc.tensor.matmul(sm_all, ocols[:, NU - u:2 * NU - u], ptacc,
                                     start=i_unit == 0, stop=i_unit == NU - 1,
                                     skip_group_check=True)
                    i_unit += 1

                    o_sb = stage.tile([D, HLOC, BLK], DT.bfloat16)
                    nc.vector.tensor_copy(out=o_sb, in_=outp)
                    nc.sync.dma_start(out=outT_d[b, tj], in_=o_sb)
            s_sb = stage.tile([NU, HLOC, BLK], mybir.dt.float32)
            nc.vector.tensor_copy(out=s_sb, in_=sm_all)
            nc.sync.dma_start(out=sums_d, in_=s_sb)
    nc.compile()
    return nc


def kernel(query, key, value, decoder_segment_ids):
    from concourse.bass_utils import run_bass_kernel_spmd

    query = np.asarray(query, dtype=np.float32)
    key = np.asarray(key, dtype=np.float32)
    value = np.asarray(value, dtype=np.float32)
    seg = np.asarray(decoder_segment_ids, dtype=np.int32)

    structs = [_segment_structure(seg[b]) for b in range(B)]
    scheds = [s[0] for s in structs]
    all_classes = [s[1] for s in structs]
    sig = tuple(
        tuple(tuple((si, tuple(ops), diag, ta, tb)
                    for (si, ops, diag, ta, tb) in entries)
              for entries in sched)
        for sched in scheds
    ) + tuple(c.tobytes() for cl in all_classes for c in cl) + (QK_MODE,)
    nc = _compiled_cache.get(sig)
    if nc is None:
        nc = _build_program(scheds, all_classes, QK_MODE)
        _compiled_cache[sig] = nc

    NU = B * NBLK
    ocols_in = np.zeros((BLK, 2 * NU), dtype=ml_dtypes.bfloat16)
    ocols_in[:, NU] = 1.0
    nbias = max(1, sum(len(c) for c in all_classes))
    bias_in = np.zeros((1, nbias * BLK), dtype=ml_dtypes.bfloat16)
    i = 0
    for cl in all_classes:
        for rows in cl:
            bias_in[0, i * BLK:(i + 1) * BLK] = np.where(rows, MASK_BIAS, 0.0)
            i += 1

    in_maps = []
    for c in range(N_CORES):
        q_c = query[:, :, c * HLOC:(c + 1) * HLOC, :]  # (B, T, HLOC, D)
        # -> (B, D, NBLK, HLOC, BLK): element [b,d,tj,h,y] = q_c[b, tj*128+y, h, d]
        qT = np.ascontiguousarray(
            q_c.transpose(0, 3, 1, 2)  # (B, D, T, HLOC)
            .reshape(B, D, NBLK, BLK, HLOC)
            .transpose(0, 1, 2, 4, 3)
        )
        kT = np.ascontiguousarray(key[:, :, c, :].transpose(0, 2, 1))  # (B, D, S)
        v_c = np.ascontiguousarray(
            value[:, :, c, :].reshape(B, NBLK, BLK, D)
        ).astype(ml_dtypes.bfloat16)
        m = {"v": v_c, "ocols_in": ocols_in, "bias_in": bias_in}
        if QK_MODE == "hilo":
            m["qhi"], m["qlo"] = _split_bf16(qT)
            m["khi"], m["klo"] = _split_bf16(kT)
        else:
            m["qhi"], m["khi"] = qT, kT
        in_maps.append(m)

    kwargs = {}
    if TRACE:
        kwargs = dict(trace=True, tmpdir=TRACE_DIR)
    res = run_bass_kernel_spmd(nc, in_maps, core_ids=list(range(N_CORES)), **kwargs)
    kernel.last_results = res

    out = np.empty((B, T, NQ, D), dtype=np.float32)
    for c in range(N_CORES):
        outT = res.results[c]["outT"]  # (B, NBLK, D, HLOC, BLK) bf16
        sums = res.results[c]["sums"]  # (B, NBLK, HLOC, BLK) f32
        o = outT.astype(np.float32).reshape(B, NBLK, D, HLOC, BLK)
        s = sums.reshape(B, NBLK, HLOC, BLK)
        # out[b, tj*128+y, c*4+h, d] = o[b, tj, d, h, y] / s[b, tj, h, y]
        o = o.transpose(0, 1, 4, 3, 2).reshape(B, T, HLOC, D)
        s = s.transpose(0, 1, 3, 2).reshape(B, T, HLOC)
        out[:, :, c * HLOC:(c + 1) * HLOC, :] = o / s[:, :, :, None]
    return out



# revision 41
# speedup vs baseline: 1.1355x; 1.0574x over previous
"""Sparse (segment + causal) GQA attention on 8 Trainium2 NeuronCores.

Problem: nn_AttentionOp_27719718928719
  query (2, 1024, 32, 128) f32, key/value (2, 1024, 8, 128) f32,
  decoder_segment_ids (2, 1024) i32 (sorted) -> out (2, 1024, 32, 128) f32

Sharding: core c owns kv-head c and its 4 GQA query heads, both batches.
Perfect shard (no replication): Q, K, V, and the output all split 8 ways, and
the compiled program is identical on every core (the block schedule depends
only on the segment ids, which all cores share).

Device algorithm, one unit per (batch b, 128-query block tj) with all 4 heads
fused along the free axis (512 wide everywhere). Per valid key block si
(causal + segment overlap, host-computed; diagonal block first, later blocks
sliced to their valid t-span):
    S^T[s, (h,t)] = K[si]^T Q      single f32r matmul (fp32 data, ~1e-3 rel)
    S^T += bias(s) * 1(h,t)        rank-1 bf16 matmuls adding -60000 to keys
                                   outside a t-span's segment (masking on PE)
    P^T = exp(S^T - 30)            ACT, writes bf16 to SBUF; the diag block
                                   writes straight into ptacc (the P-sum
                                   accumulator), later blocks into pt tiles
    causal zero (diag blocks only) one Pool affine_select for all 4 heads
    outT[d, (h,t)] += V[si]' P^T   bf16 matmul, PSUM-accumulated over si
    ptacc += P^T                   DVE add (keeps sums work off the PE)
  per unit: one eye-column matmul drops colsum(ptacc) into partition row u of
  a single shared [16, 512] PSUM bank (one sums drain for the whole kernel);
  outT drains PSUM -> SBUF as bf16 (DVE) then DMAs out.
No softmax max-subtraction: logits are O(+-50) so exp(x-30) stays in range and
exp(x)/sum(exp(x)) matches the reference's exp(x-max)/sum(exp(x-max)) exactly.
Host does the (cheap) normalization out/sums and all layout transposes.
DMA: only b=0's working set is enqueued up front (gating chunks first, one
per DMA-capable engine); b=1's loads interleave into b=0's compute stream.
"""

import numpy as np
import ml_dtypes

B, T, S, NQ, NKV, D = 2, 1024, 1024, 32, 8, 128
G = NQ // NKV
BLK = 128
NBLK = S // BLK  # 8
W = G * BLK  # 512: fused 4-head free width
N_CORES = 8
HLOC = NQ // N_CORES  # 4
MASK_BIAS = -60000.0

_compiled_cache = {}

# Test-only knobs (the grading path never sets these): when TRACE is true the
# SPMD run captures an NTFF profile into TRACE_DIR.
TRACE = False
TRACE_DIR = None

# QK product mode: "hilo" = 3 bf16 hi/lo matmuls (fp32-grade logits),
# "f32r" = single fp32r matmul (3x less PE time, ~30x more logit error).
QK_MODE = "f32r"


def _split_bf16(x):
    hi = x.astype(ml_dtypes.bfloat16)
    lo = (x - hi.astype(np.float32)).astype(ml_dtypes.bfloat16)
    return hi, lo


def _segment_structure(seg):
    """Block schedule for one batch's (sorted) segment ids.

    Returns (sched, bias_classes):
      sched[tj] = list of (si, bias_ops, diag) where bias_ops is a list of
        (a, b, cls) adding bias class `cls` to t-columns [a, b) of the block,
        and diag marks the causal in-block mask.
      bias_classes = list of np bool arrays [BLK]: True where the key row gets
        MASK_BIAS.
    """
    seg = np.asarray(seg)
    t_idx = np.arange(S)
    seg_start = np.zeros(S, np.int64)
    seg_end = np.zeros(S, np.int64)
    for v in np.unique(seg):
        m = seg == v
        lo, hi = np.argmax(m), S - np.argmax(m[::-1])
        seg_start[m], seg_end[m] = lo, hi
    valid_ts = (t_idx[None, :] <= t_idx[:, None]) & (seg[None, :] == seg[:, None])
    v4 = valid_ts.reshape(NBLK, BLK, NBLK, BLK)
    vblk = v4.any(axis=(1, 3))  # [tj, si]
    fblk = v4.all(axis=(1, 3))

    classes = []  # list of np.bool arrays
    cls_key = {}

    def class_id(mask_rows):
        key = mask_rows.tobytes()
        if key not in cls_key:
            cls_key[key] = len(classes)
            classes.append(mask_rows.copy())
        return cls_key[key]

    sched = []
    for tj in range(NBLK):
        entries = []
        sis = [si for si in range(NBLK) if vblk[tj, si]]
        assert sis == list(range(min(sis), max(sis) + 1))
        for si in sis:
            # valid t-span [ta, tb): t-columns with at least one valid key in
            # this block. Outside it every key row is masked, exp == 0.0
            # exactly, so the whole column is skipped in QK/exp/PV/sums.
            tvalid = v4[tj, :, si, :].any(axis=1)
            w = np.where(tvalid)[0]
            ta, tb = int(w[0]), int(w[-1]) + 1
            assert (np.diff(w) == 1).all(), "t-span not contiguous"
            bias_ops = []
            if not fblk[tj, si] and not (si == tj and _only_causal(v4, tj, si)):
                tcols = np.arange(tj * BLK, (tj + 1) * BLK)
                lo_rel = np.clip(seg_start[tcols] - si * BLK, 0, BLK)
                hi_rel = np.clip(seg_end[tcols] - si * BLK, 0, BLK)
                a = ta
                for i in range(ta + 1, tb + 1):
                    if i == tb or lo_rel[i] != lo_rel[a] or hi_rel[i] != hi_rel[a]:
                        lo, hi = int(lo_rel[a]), int(hi_rel[a])
                        rows = np.ones(BLK, dtype=bool)
                        rows[lo:hi] = False  # False -> keep
                        if rows.any():
                            bias_ops.append((a, i, class_id(rows)))
                        a = i
            entries.append((si, bias_ops, si == tj, ta, tb))
        # diag block first: it always spans the full t-range, so it carries
        # the start=True PSUM init for outp/sums; later (possibly t-sliced)
        # blocks accumulate into an already-initialized region.
        assert entries[-1][2], "diag must be the last ascending entry"
        entries = [entries[-1]] + entries[:-1]
        sched.append(entries)
    return sched, classes


def _only_causal(v4, tj, si):
    """True if block (tj, si)'s invalid entries are exactly the causal ones."""
    blk = v4[tj, :, si, :]  # [t, s]
    t = np.arange(BLK)[:, None] + tj * BLK
    s = np.arange(BLK)[None, :] + si * BLK
    return bool((blk == (s <= t)).all())


def _build_program(scheds, all_classes, qk_mode):
    """Build the SPMD Bass program. scheds/all_classes indexed by batch."""
    import concourse.bass as bass  # noqa: F401
    from concourse import bacc
    import concourse.mybir as mybir
    import concourse.tile as tile

    DT = mybir.dt
    F32R = DT.float32r
    QDT = DT.bfloat16 if qk_mode == "hilo" else F32R
    ncls = [len(c) for c in all_classes]
    nc = bacc.Bacc(None, target_bir_lowering=False, debug=False)

    qhi_d = nc.dram_tensor("qhi", [B, D, NBLK, HLOC, BLK], QDT, kind="ExternalInput").ap()
    khi_d = nc.dram_tensor("khi", [B, D, S], QDT, kind="ExternalInput").ap()
    if qk_mode == "hilo":
        qlo_d = nc.dram_tensor("qlo", [B, D, NBLK, HLOC, BLK], QDT, kind="ExternalInput").ap()
        klo_d = nc.dram_tensor("klo", [B, D, S], QDT, kind="ExternalInput").ap()
    v_d = nc.dram_tensor("v", [B, NBLK, BLK, D], DT.bfloat16, kind="ExternalInput").ap()
    NU = B * NBLK  # 16 (b, tj) units; each owns one partition row of sm_all
    ocols_d = nc.dram_tensor("ocols_in", [BLK, 2 * NU], DT.bfloat16, kind="ExternalInput").ap()
    nbias = max(1, sum(ncls))
    bias_d = nc.dram_tensor("bias_in", [1, nbias * BLK], DT.bfloat16, kind="ExternalInput").ap()
    outT_d = nc.dram_tensor(
        "outT", [B, NBLK, D, HLOC, BLK], DT.bfloat16, kind="ExternalOutput"
    ).ap()
    sums_d = nc.dram_tensor(
        "sums", [NU, HLOC, BLK], DT.float32, kind="ExternalOutput"
    ).ap()

    cls_base = [0, ncls[0]]  # class index offset per batch

    with tile.TileContext(nc) as tc:
        with (
            tc.tile_pool(name="const", bufs=1) as constp,
            tc.tile_pool(name="qkv", bufs=1) as qkv,
            tc.tile_pool(name="pt", bufs=6) as ptp,
            tc.tile_pool(name="pacc", bufs=3) as paccp,
            tc.tile_pool(name="stage", bufs=4) as stage,
            tc.tile_pool(name="ps_s", bufs=4, space="PSUM") as ps_s,
            tc.tile_pool(name="ps_o", bufs=3, space="PSUM") as ps_o,
            tc.tile_pool(name="ps_m", bufs=1, space="PSUM") as ps_m,
        ):
            # b=0 inputs first so compute can start while b=1 still loads
            k_hi = qkv.tile([D, B, S], QDT)
            v_t = qkv.tile([BLK, B, NBLK, D], DT.bfloat16)
            q_hi = qkv.tile([D, B, NBLK, HLOC, BLK], QDT)
            if qk_mode == "hilo":
                k_lo = qkv.tile([D, B, S], QDT)
                q_lo = qkv.tile([D, B, NBLK, HLOC, BLK], QDT)
            # ocols[:, 16-u:32-u] is an [128, 16] eye-column window (column u
            # all-ones, rest zero) -> the sums matmul for unit u lands its
            # column sums in partition row u of the shared sm_all tile.
            ocols = constp.tile([BLK, 2 * NU], DT.bfloat16)
            ones_bf = constp.tile([1, HLOC, BLK], DT.bfloat16)
            bias_t = constp.tile([1, nbias * BLK], DT.bfloat16)
            exp_bias = constp.tile([BLK, 1], mybir.dt.float32)
            # All 16 units' sums accumulate in one PSUM bank (one drain at
            # the very end); each unit contributes one 512-row matmul over
            # its DVE-accumulated ptacc instead of one matmul per block.
            sm_all = ps_m.tile([NU, HLOC, BLK], mybir.dt.float32)

            def load_q(b, lo_blk, hi_blk, eng=None):
                eng = eng or nc.sync
                sl = np.s_[lo_blk:hi_blk]
                if qk_mode == "hilo":
                    eng.dma_start(out=q_hi[:, b, sl], in_=qhi_d[b, :, sl])
                    eng.dma_start(out=q_lo[:, b, sl], in_=qlo_d[b, :, sl])
                else:
                    eng.dma_start(
                        out=q_hi[:, b, sl], in_=qhi_d[b, :, sl].bitcast(F32R)
                    )

            def load_k(b, lo_blk, hi_blk, eng):
                sl = np.s_[lo_blk * BLK:hi_blk * BLK]
                if qk_mode == "hilo":
                    eng.dma_start(out=k_hi[:, b, sl], in_=khi_d[b, :, sl])
                    eng.dma_start(out=k_lo[:, b, sl], in_=klo_d[b, :, sl])
                else:
                    eng.dma_start(
                        out=k_hi[:, b, sl], in_=khi_d[b, :, sl].bitcast(F32R)
                    )

            def load_v(b, lo_blk, hi_blk, eng):
                sl = np.s_[lo_blk:hi_blk]
                eng.dma_start(
                    out=v_t[:, b, sl],
                    in_=v_d[b, sl].rearrange("si p d -> p si d"),
                )

            # The first unit (b0, tj0) needs only k/q/v block 0; stage those
            # tiny gating chunks first, one per DMA-capable engine (sync,
            # scalar, gpsimd) so all three hit the DMA queues immediately.
            # Only b=0's working set is enqueued up front so it doesn't share
            # DMA bandwidth with b=1 data that isn't needed until much later;
            # b=1's loads are interleaved into b=0's compute stream below.
            load_q(0, 0, 1, nc.scalar)
            load_k(0, 0, 2, nc.sync)
            load_v(0, 0, 2, nc.gpsimd)
            load_k(0, 2, NBLK, nc.scalar)
            nc.sync.dma_start(out=bias_t, in_=bias_d)
            nc.sync.dma_start(out=ocols, in_=ocols_d)
            nc.vector.memset(ones_bf, 1.0)
            nc.vector.memset(exp_bias, -30.0)
            load_q(0, 1, 2)
            load_v(0, 2, NBLK, nc.gpsimd)
            load_q(0, 2, 4)
            load_q(0, 4, NBLK)
            # b=1 loads, issued on sync between b=0 unit outputs (see loop)
            deferred = {
                1: lambda: load_k(1, 0, NBLK, nc.sync),
                2: lambda: load_q(1, 0, 2),
                3: lambda: load_q(1, 2, 4),
                4: lambda: load_q(1, 4, 6),
                5: lambda: (load_q(1, 6, NBLK), load_v(1, 0, NBLK, nc.sync)),
            }

            i_unit = 0
            for b in range(B):
                # b=1 runs its lightest unit (tj0: diag only) last so the
                # final pipeline drain is as short as possible; b=0 keeps
                # ascending order to match the input DMA arrival order.
                tj_order = list(range(NBLK)) if b == 0 else list(range(1, NBLK)) + [0]
                for tj in tj_order:
                    if b == 0 and tj in deferred:
                        deferred[tj]()
                    entries = scheds[b][tj]
                    u = b * NBLK + tj
                    outp = ps_o.tile([D, HLOC, BLK], mybir.dt.float32)
                    ptacc = paccp.tile([BLK, HLOC, BLK], DT.bfloat16)
                    n_e = len(entries)
                    for idx, (si, bias_ops, diag, ta, tb) in enumerate(entries):
                        st = ps_s.tile([BLK, HLOC, BLK], mybir.dt.float32)
                        kh = k_hi[:, b, si * BLK:(si + 1) * BLK]
                        qh = q_hi[:, b, tj]
                        tsl = np.s_[ta:tb]
                        # f32r matmuls below 256 moving rows run at 1/4 rate;
                        # pad the QK t-span to >= 64 columns (4 heads x 64 =
                        # 256 rows). Padded columns compute garbage logits
                        # that exp/PV/sums never read.
                        qa, qb = ta, tb
                        if qk_mode != "hilo" and qb - qa < 64:
                            qa = max(0, min(ta, BLK - 64))
                            qb = qa + 64
                        qsl = np.s_[qa:qb]
                        last_qk = len(bias_ops) == 0
                        if qk_mode == "hilo":
                            kl = k_lo[:, b, si * BLK:(si + 1) * BLK]
                            ql = q_lo[:, b, tj]
                            nc.tensor.matmul(st[:, :, tsl], kh, qh[:, :, tsl],
                                             start=True, stop=False,
                                             skip_group_check=True)
                            nc.tensor.matmul(st[:, :, tsl], kh, ql[:, :, tsl],
                                             start=False, stop=False,
                                             skip_group_check=True)
                            nc.tensor.matmul(st[:, :, tsl], kl, qh[:, :, tsl],
                                             start=False, stop=last_qk,
                                             skip_group_check=True)
                        else:
                            nc.tensor.matmul(st[:, :, qsl], kh, qh[:, :, qsl],
                                             start=True, stop=last_qk,
                                             skip_group_check=True)
                        for bi, (a, e, cls) in enumerate(bias_ops):
                            cid = cls_base[b] + cls
                            nc.tensor.matmul(
                                st[:, :, a:e],
                                bias_t[:, cid * BLK:(cid + 1) * BLK],
                                ones_bf[:, :, :e - a],
                                start=False, stop=bi == len(bias_ops) - 1,
                                skip_group_check=True,
                            )

                        # exp(x - 30): headroom against fp32 exp overflow for
                        # unlucky logit maxima; cancels in out/sums exactly.
                        # The diag block (always first, always full-width)
                        # writes straight into ptacc, which doubles as its
                        # pt: no init copy needed.
                        if idx == 0:
                            pt = ptacc
                        else:
                            pt = ptp.tile([BLK, HLOC, BLK], DT.bfloat16)
                        nc.scalar.activation(
                            out=pt[:, :, tsl], in_=st[:, :, tsl],
                            func=mybir.ActivationFunctionType.Exp,
                            bias=exp_bias,
                        )
                        if diag:
                            # keep s <= t for every head: iota = -4x + h + 4y,
                            # >= 0 iff y >= x (h in 0..3 can't flip it)
                            nc.gpsimd.affine_select(
                                out=pt, in_=pt, compare_op=mybir.AluOpType.is_ge,
                                fill=0.0, base=0,
                                pattern=[[1, HLOC], [HLOC, BLK]],
                                channel_multiplier=-HLOC,
                            )

                        # diag runs first and spans the full t-range, so it
                        # initializes the PSUM region the t-sliced later
                        # blocks accumulate into.
                        nc.tensor.matmul(outp[:, :, tsl], v_t[:, b, si],
                                         pt[:, :, tsl],
                                         start=idx == 0, stop=idx == n_e - 1,
                                         skip_group_check=True)
                        # accumulate P on DVE (bf16 2x mode) so the PE pays
                        # one 512-row sums matmul per unit instead of one
                        # per block; the diag exp already initialized ptacc.
                        if idx > 0:
                            nc.vector.tensor_add(
                                ptacc[:, :, tsl], pt[:, :, tsl], ptacc[:, :, tsl]
                            )

                    nc.tensor.matmul(sm_all, ocols[:, NU - u:2 * NU - u], ptacc,
                                     start=i_unit == 0, stop=i_unit == NU - 1,
                                     skip_group_check=True)
                    i_unit += 1

                    o_sb = stage.tile([D, HLOC, BLK], DT.bfloat16)
                    nc.vector.tensor_copy(out=o_sb, in_=outp)
                    nc.sync.dma_start(out=outT_d[b, tj], in_=o_sb)
            s_sb = stage.tile([NU, HLOC, BLK], mybir.dt.float32)
            nc.vector.tensor_copy(out=s_sb, in_=sm_all)
            nc.sync.dma_start(out=sums_d, in_=s_sb)
    nc.compile()
    return nc


def kernel(query, key, value, decoder_segment_ids):
    from concourse.bass_utils import run_bass_kernel_spmd

    query = np.asarray(query, dtype=np.float32)
    key = np.asarray(key, dtype=np.float32)
    value = np.asarray(value, dtype=np.float32)
    seg = np.asarray(decoder_segment_ids, dtype=np.int32)

    structs = [_segment_structure(seg[b]) for b in range(B)]
    scheds = [s[0] for s in structs]
    all_classes = [s[1] for s in structs]
    sig = tuple(
        tuple(tuple((si, tuple(ops), diag, ta, tb)
                    for (si, ops, diag, ta, tb) in entries)
              for entries in sched)
        for sched in scheds
    ) + tuple(c.tobytes() for cl in all_classes for c in cl) + (QK_MODE,)
    nc = _compiled_cache.get(sig)
    if nc is None:
        nc = _build_program(scheds, all_classes, QK_MODE)
        _compiled_cache[sig] = nc

    NU = B * NBLK
    ocols_in = np.zeros((BLK, 2 * NU), dtype=ml_dtypes.bfloat16)
    ocols_in[:, NU] = 1.0
    nbias = max(1, sum(len(c) for c in all_classes))
    bias_in = np.zeros((1, nbias * BLK), dtype=ml_dtypes.bfloat16)
    i = 0
    for cl in all_classes:
        for rows in cl:
            bias_in[0, i * BLK:(i + 1) * BLK] = np.where(rows, MASK_BIAS, 0.0)
            i += 1

    in_maps = []
    for c in range(N_CORES):
        q_c = query[:, :, c * HLOC:(c + 1) * HLOC, :]  # (B, T, HLOC, D)
        # -> (B, D, NBLK, HLOC, BLK): element [b,d,tj,h,y] = q_c[b, tj*128+y, h, d]
        qT = np.ascontiguousarray(
            q_c.transpose(0, 3, 1, 2)  # (B, D, T, HLOC)
            .reshape(B, D, NBLK, BLK, HLOC)
            .transpose(0, 1, 2, 4, 3)
        )
        kT = np.ascontiguousarray(key[:, :, c, :].transpose(0, 2, 1))  # (B, D, S)
        v_c = np.ascontiguousarray(
            value[:, :, c, :].reshape(B, NBLK, BLK, D)
        ).astype(ml_dtypes.bfloat16)
        m = {"v": v_c, "ocols_in": ocols_in, "bias_in": bias_in}
        if QK_MODE == "hilo":
            m["qhi"], m["qlo"] = _split_bf16(qT)
            m["khi"], m["klo"] = _split_bf16(kT)
        else:
            m["qhi"], m["khi"] = qT, kT
        in_maps.append(m)

    kwargs = {}
    if TRACE:
        kwargs = dict(trace=True, tmpdir=TRACE_DIR)
    res = run_bass_kernel_spmd(nc, in_maps, core_ids=list(range(N_CORES)), **kwargs)
    kernel.last_results = res

    out = np.empty((B, T, NQ, D), dtype=np.float32)
    for c in range(N_CORES):
        outT = res.results[c]["outT"]  # (B, NBLK, D, HLOC, BLK) bf16
        sums = res.results[c]["sums"]  # (B, NBLK, HLOC, BLK) f32
        o = outT.astype(np.float32).reshape(B, NBLK, D, HLOC, BLK)
        s = sums.reshape(B, NBLK, HLOC, BLK)
        # out[b, tj*128+y, c*4+h, d] = o[b, tj, d, h, y] / s[b, tj, h, y]
        o = o.transpose(0, 1, 4, 3, 2).reshape(B, T, HLOC, D)
        s = s.transpose(0, 1, 3, 2).reshape(B, T, HLOC)
        out[:, :, c * HLOC:(c + 1) * HLOC, :] = o / s[:, :, :, None]
    return out

